# revision 21
# baseline (speedup 1.0000x reference)
"""Trainium2 Bass kernel for AttentionalPlanarRemapping.

out[n,c,h,w] = sum_d softmax(atts[n,c,:])[d] * images[n,d,h,w]

Per-sample: W = softmax(atts[n]) [C,C]; out[n] = W @ images[n].reshape(C, H*W).

Sharding: data-parallel over N across 8 cores (4 samples per core).

Host preprocessing inside kernel(): atts is passed TRANSPOSED per sample
(attsT[n] = atts[n].T, layout [d, c]) and converted to fp16, so attsT loads
with the contraction dim d on partitions (the matmul lhsT layout) at half
the DMA cost. images are uploaded fp16 and the output stored fp16: the
rel-err budget (2e-2) dwarfs fp16 rounding, and HBM bandwidth is the
co-bottleneck with the PE (10MB DMA ~28us vs 27.5us of fp16 matmul).

Per-core structure (pipelined one sample ahead):
  prep_io(n): one whole-sample DMA per tensor (the sync sequencer's
    DIRECT2D issue cost ~0.6us is per-dma_start, so chunked loads
    serialize on issue), then E = exp(attsT) per kd-block on ACT.
  prep_T(n):  T = sum_kd E (3 DVE adds) -- the free-axis half of the
    softmax denominator.
  compute(n): 4 tiny matmuls T_blk.T @ ones[128,2] finish the denominator
    (partition-sum) directly in per-partition layout; r = 1/s (DVE); then
    per kc: 8 accumulating matmuls into psum, evict psum -> O fp16 scaled
    by r[kc] (alternating ACT/DVE), store per kc band (alternating
    SWDGE/ACT-HWDGE queues so stores never block the sync load queue).
    The very last band's eviction is split ACT/DVE in parallel with its
    two half-stores on the gpsimd/sync queues (shortest post-matmul tail;
    PSUM dependencies are tile-granular, so an eviction can never overlap
    later matmuls into the same psum tile).

Emission interleaves prep(n+1) into compute(n) so the ACT queue runs
[evict(n,kc0), exps(n+1), evict(n,kc2)]: an eviction is never queued
behind exps that wait on a DMA, which would hold PSUM banks hostage and
stall the PE (engines execute their instruction streams in FIFO order).

No PE warmup stream: the HAM clock gate needs ~3.4us of CONTINUOUS PE
activity to lift, which cannot fit before the first loads land (~10us),
and a busy PE delays DMA completion semaphores ~1:1 (SBUF port
contention), starving the exp chain. The first ~10 matmuls run at
1.2GHz; that is unavoidable.
"""

import numpy as np
from contextlib import ExitStack

import concourse.bass as bass
import concourse.mybir as mybir
import concourse.tile as tile
from concourse import bacc
from concourse.bass_utils import run_bass_kernel_spmd

N, C, H, W = 32, 512, 32, 32
HW = H * W                      # 1024
NCORES = 8
NPC = N // NCORES               # 4 samples per core
P = 128
KC = C // P                     # 4 chunks over output channel c
KD = C // P                     # 4 chunks over contraction d
NT = 512                        # matmul moving free dim (one PSUM bank of f32)
NHT = HW // NT                  # 2

F32 = mybir.dt.float32
F16 = mybir.dt.float16
AF = mybir.ActivationFunctionType
OP = mybir.AluOpType


def build_nc():
    nc = bacc.Bacc("TRN2", target_bir_lowering=False, debug=False)

    images = nc.dram_tensor("images", [NPC, C, HW], F16, kind="ExternalInput").ap()
    attsT = nc.dram_tensor("attsT", [NPC, C, C], F16, kind="ExternalInput").ap()
    out = nc.dram_tensor("out", [NPC, C, HW], F16, kind="ExternalOutput").ap()

    with ExitStack() as ctx:
        tc = ctx.enter_context(tile.TileContext(nc))

        const_pool = ctx.enter_context(tc.tile_pool(name="const", bufs=1))
        ones2_f32 = const_pool.tile([P, 2], F32)
        ones2 = const_pool.tile([P, 2], F16)

        a_pool = ctx.enter_context(tc.tile_pool(name="a", bufs=2))
        e_pool = ctx.enter_context(tc.tile_pool(name="e", bufs=2))
        t_pool = ctx.enter_context(tc.tile_pool(name="t", bufs=2))
        x_pool = ctx.enter_context(tc.tile_pool(name="x", bufs=3))
        o_pool = ctx.enter_context(tc.tile_pool(name="o", bufs=2))
        r_pool = ctx.enter_context(tc.tile_pool(name="r", bufs=2))
        sm_psum = ctx.enter_context(tc.tile_pool(name="smp", bufs=1, space="PSUM"))
        mm_psum = ctx.enter_context(tc.tile_pool(name="mmp", bufs=3, space="PSUM"))

        nc.vector.memset(ones2_f32[:], 1.0)
        nc.vector.tensor_copy(ones2[:], ones2_f32[:])

        def prep_io(n):
            """Whole-sample input DMAs + exp for sample n."""
            a_t = a_pool.tile([P, KD, C], F16, name=f"a{n}", tag="a")
            x_t = x_pool.tile([P, KD, HW], F16, name=f"x{n}", tag="x")
            nc.sync.dma_start(
                a_t[:], attsT[n].rearrange("(kd p) c -> p kd c", p=P)
            )
            if n == 0:
                for h in range(2):
                    nc.sync.dma_start(
                        x_t[:, h * 2 : (h + 1) * 2],
                        images[n][h * 256 : (h + 1) * 256].rearrange(
                            "(kd p) f -> p kd f", p=P
                        ),
                    )
            else:
                nc.sync.dma_start(
                    x_t[:], images[n].rearrange("(kd p) f -> p kd f", p=P)
                )
            e_t = e_pool.tile([P, KD, C], F16, name=f"e{n}", tag="e")
            for kd in range(KD):
                nc.scalar.activation(
                    e_t[:, kd], a_t[:, kd], AF.Exp, bias=0.0, scale=1.0
                )
            return e_t, x_t

        def prep_T(n, e_t):
            """T[d_p, c] = sum_kd E[d_p, kd, c] (DVE): free-axis half of the
            softmax denominator; the partition half happens in tiny matmuls."""
            t2 = t_pool.tile([P, 2, C], F16, name=f"t2_{n}", tag="t2")
            nc.vector.scalar_tensor_tensor(
                t2[:, 0], e_t[:, 0], 1.0, e_t[:, 1], op0=OP.mult, op1=OP.add
            )
            nc.vector.scalar_tensor_tensor(
                t2[:, 1], e_t[:, 2], 1.0, e_t[:, 3], op0=OP.mult, op1=OP.add
            )
            tsum = t_pool.tile([P, C], F16, name=f"ts{n}", tag="ts")
            nc.vector.scalar_tensor_tensor(
                tsum[:], t2[:, 0], 1.0, t2[:, 1], op0=OP.mult, op1=OP.add
            )
            return tsum

        def denom(n, tsum):
            """s[c] = sum_p T[p, c] via tiny matmuls (lands the softmax
            denominator directly on the output-channel partitions); r = 1/s."""
            rp_ps = sm_psum.tile(
                [P, 2 * KC], F32, name=f"rp{n}", tag="rp", space="PSUM"
            )
            for j in range(KC):
                nc.tensor.matmul(
                    rp_ps[:, j * 2 : (j + 1) * 2],
                    lhsT=tsum[:, j * P : (j + 1) * P],
                    rhs=ones2[:],
                )
            s_col = r_pool.tile([P, KC], F32, name=f"scol{n}", tag="scol")
            nc.vector.tensor_copy(
                s_col[:],
                rp_ps[:].rearrange("p (kc j) -> p kc j", j=2)[:, :, 0],
            )
            r_sb = r_pool.tile([P, KC], F32, name=f"rsb{n}", tag="rsb")
            nc.vector.reciprocal(r_sb[:], s_col[:])
            return r_sb

        def mm_band(ps, e_t, x_t, kc, ht_list):
            for kd in range(KD):
                for ht in ht_list:
                    nc.tensor.matmul(
                        ps[:, ht * NT : (ht + 1) * NT],
                        lhsT=e_t[:, kd, kc * P : (kc + 1) * P],
                        rhs=x_t[:, kd, ht * NT : (ht + 1) * NT],
                        start=(kd == 0),
                        stop=(kd == KD - 1),
                    )

        def compute(n, e_t, x_t, tsum, next_io, next_T):
            """next_io/next_T: callbacks emitting the next sample's prep at
            queue positions that keep evictions ahead of dependent exps."""
            last = n == NPC - 1
            r_sb = None if n == 0 else denom(n, tsum)
            for kc in range(KC):
                ps = mm_psum.tile(
                    [P, HW], F32, name=f"ps{n}_{kc}", tag="ps", space="PSUM"
                )
                mm_band(ps, e_t, x_t, kc, range(NHT))
                if kc == 0 and n == 0:
                    r_sb = denom(n, tsum)
                o_t = o_pool.tile([P, HW], F16, name=f"o{n}_{kc}", tag=f"o{kc}")
                r_ap = r_sb[:, kc : kc + 1]
                if last and kc == KC - 1:
                    # tail: split the final eviction across ACT+DVE and put
                    # the final half-store on the idle sync queue, whose
                    # sequencer isn't busy running the other eviction
                    nc.scalar.mul(o_t[:, :NT], ps[:, :NT], r_ap)
                    nc.vector.tensor_scalar_mul(o_t[:, NT:], ps[:, NT:], r_ap)
                    nc.gpsimd.dma_start(
                        out[n][kc * P : (kc + 1) * P, :NT], o_t[:, :NT]
                    )
                    nc.sync.dma_start(
                        out[n][kc * P : (kc + 1) * P, NT:], o_t[:, NT:]
                    )
                    continue
                # eviction engines: ACT on even kc, DVE on odd. For the last
                # sample, kc0 goes to the otherwise-idle DVE so the psum
                # slot the final band reuses is freed early, and the DVE is
                # kept clear of kc2 so the final split eviction isn't queued
                # behind a 1.3us whole-band eviction.
                act_evict = (kc % 2 == 0) if not last else (kc in (1, 2))
                if act_evict:
                    nc.scalar.mul(o_t[:], ps[:], r_ap)
                    nc.gpsimd.dma_start(out[n][kc * P : (kc + 1) * P], o_t[:])
                else:
                    nc.vector.tensor_scalar_mul(o_t[:], ps[:], r_ap)
                    nc.scalar.dma_start(out[n][kc * P : (kc + 1) * P], o_t[:])
                if kc == 0 and next_io is not None:
                    next_io()
                if kc == 1 and next_T is not None:
                    next_T()

        # software pipeline: sample n+1's loads/exps are emitted inside
        # compute(n) right after evict(kc0) so ACT never holds a PSUM bank
        # hostage behind a DMA-gated exp
        state = {}
        state[0] = prep_io(0)
        t0 = prep_T(0, state[0][0])

        def mk_io(m):
            def f():
                state[m] = prep_io(m)
            return f

        def mk_T(m):
            def f():
                state[m] = (*state[m], prep_T(m, state[m][0]))
            return f

        cur_T = t0
        for n in range(NPC):
            e_t, x_t = state[n][0], state[n][1]
            nio = mk_io(n + 1) if n + 1 < NPC else None
            nT = mk_T(n + 1) if n + 1 < NPC else None
            compute(n, e_t, x_t, cur_T, nio, nT)
            if n + 1 < NPC:
                cur_T = state[n + 1][2]

    nc.compile()
    return nc


_NC_CACHE = None


def _get_nc():
    global _NC_CACHE
    if _NC_CACHE is None:
        _NC_CACHE = build_nc()
    return _NC_CACHE


def run(in_maps, **kwargs):
    """Run the SPMD kernel on cores 0..7. in_maps: one dict per core."""
    nc = _get_nc()
    return run_bass_kernel_spmd(nc, in_maps, core_ids=list(range(NCORES)), **kwargs)


def make_in_maps(images: np.ndarray, atts: np.ndarray):
    images = np.asarray(images, dtype=np.float32).astype(np.float16)
    atts = np.asarray(atts, dtype=np.float32)
    assert images.shape == (N, C, H, W), images.shape
    assert atts.shape == (N, C, C), atts.shape
    img_s = images.reshape(NCORES, NPC, C, HW)
    # per-sample transpose: attsT[n] = atts[n].T  (layout [d, c])
    attsT = np.ascontiguousarray(atts.transpose(0, 2, 1)).astype(np.float16)
    attsT = attsT.reshape(NCORES, NPC, C, C)
    return [
        {"images": np.ascontiguousarray(img_s[i]), "attsT": attsT[i]}
        for i in range(NCORES)
    ]


def kernel(images: np.ndarray, atts: np.ndarray) -> np.ndarray:
    in_maps = make_in_maps(images, atts)
    res = run(in_maps)
    outs = [res.results[i]["out"] for i in range(NCORES)]
    full = np.concatenate(outs, axis=0).reshape(N, C, H, W)
    return full.astype(np.float32)


# revision 23
# speedup vs baseline: 1.0070x; 1.0070x over previous
"""Trainium2 Bass kernel for AttentionalPlanarRemapping.

out[n,c,h,w] = sum_d softmax(atts[n,c,:])[d] * images[n,d,h,w]

Per-sample: W = softmax(atts[n]) [C,C]; out[n] = W @ images[n].reshape(C, H*W).

Sharding: data-parallel over N across 8 cores (4 samples per core).

Host preprocessing inside kernel(): atts is passed TRANSPOSED per sample
(attsT[n] = atts[n].T, layout [d, c]) and converted to fp16, so attsT loads
with the contraction dim d on partitions (the matmul lhsT layout) at half
the DMA cost. images are uploaded fp16 and the output stored fp16: the
rel-err budget (2e-2) dwarfs fp16 rounding, and HBM bandwidth is the
co-bottleneck with the PE (10MB DMA ~28us vs 27.5us of fp16 matmul).

Per-core structure (pipelined one sample ahead):
  prep_io(n): one whole-sample DMA per tensor (the sync sequencer's
    DIRECT2D issue cost ~0.6us is per-dma_start, so chunked loads
    serialize on issue), then E = exp(attsT) per kd-block on ACT.
  prep_T(n):  T = sum_kd E (3 DVE adds) -- the free-axis half of the
    softmax denominator.
  compute(n): 4 tiny matmuls T_blk.T @ ones[128,2] finish the denominator
    (partition-sum) directly in per-partition layout; r = 1/s (DVE); then
    per kc: 8 accumulating matmuls into psum, evict psum -> O fp16 scaled
    by r[kc] (alternating ACT/DVE), store per kc band (alternating
    SWDGE/ACT-HWDGE queues so stores never block the sync load queue).
    The very last band's eviction is split ACT/DVE in parallel with its
    two half-stores on the gpsimd/sync queues (shortest post-matmul tail;
    PSUM dependencies are tile-granular, so an eviction can never overlap
    later matmuls into the same psum tile).

Emission interleaves prep(n+1) into compute(n) so the ACT queue runs
[evict(n,kc0), exps(n+1), evict(n,kc2)]: an eviction is never queued
behind exps that wait on a DMA, which would hold PSUM banks hostage and
stall the PE (engines execute their instruction streams in FIFO order).

No PE warmup stream: the HAM clock gate needs ~3.4us of CONTINUOUS PE
activity to lift, which cannot fit before the first loads land (~10us),
and a busy PE delays DMA completion semaphores ~1:1 (SBUF port
contention), starving the exp chain. The first ~10 matmuls run at
1.2GHz; that is unavoidable.
"""

import numpy as np
from contextlib import ExitStack

import concourse.bass as bass
import concourse.mybir as mybir
import concourse.tile as tile
from concourse import bacc
from concourse.bass_utils import run_bass_kernel_spmd

N, C, H, W = 32, 512, 32, 32
HW = H * W                      # 1024
NCORES = 8
NPC = N // NCORES               # 4 samples per core
P = 128
KC = C // P                     # 4 chunks over output channel c
KD = C // P                     # 4 chunks over contraction d
NT = 512                        # matmul moving free dim (one PSUM bank of f32)
NHT = HW // NT                  # 2

F32 = mybir.dt.float32
F16 = mybir.dt.float16
AF = mybir.ActivationFunctionType
OP = mybir.AluOpType


def build_nc():
    nc = bacc.Bacc("TRN2", target_bir_lowering=False, debug=False)

    images = nc.dram_tensor("images", [NPC, C, HW], F16, kind="ExternalInput").ap()
    attsT = nc.dram_tensor("attsT", [NPC, C, C], F16, kind="ExternalInput").ap()
    out = nc.dram_tensor("out", [NPC, C, HW], F16, kind="ExternalOutput").ap()

    with ExitStack() as ctx:
        tc = ctx.enter_context(tile.TileContext(nc))

        const_pool = ctx.enter_context(tc.tile_pool(name="const", bufs=1))
        ones2_f32 = const_pool.tile([P, 2], F32)
        ones2 = const_pool.tile([P, 2], F16)

        a_pool = ctx.enter_context(tc.tile_pool(name="a", bufs=2))
        e_pool = ctx.enter_context(tc.tile_pool(name="e", bufs=2))
        t_pool = ctx.enter_context(tc.tile_pool(name="t", bufs=2))
        x_pool = ctx.enter_context(tc.tile_pool(name="x", bufs=3))
        o_pool = ctx.enter_context(tc.tile_pool(name="o", bufs=2))
        r_pool = ctx.enter_context(tc.tile_pool(name="r", bufs=2))
        sm_psum = ctx.enter_context(tc.tile_pool(name="smp", bufs=1, space="PSUM"))
        mm_psum = ctx.enter_context(tc.tile_pool(name="mmp", bufs=3, space="PSUM"))

        nc.vector.memset(ones2_f32[:], 1.0)
        nc.vector.tensor_copy(ones2[:], ones2_f32[:])

        def prep_io(n):
            """Whole-sample input DMAs + exp for sample n."""
            a_t = a_pool.tile([P, KD, C], F16, name=f"a{n}", tag="a")
            x_t = x_pool.tile([P, KD, HW], F16, name=f"x{n}", tag="x")
            e_t = e_pool.tile([P, KD, C], F16, name=f"e{n}", tag="e")
            if n == 0:
                # entry is load-bandwidth-bound: interleave kd-halves of the
                # two tensors so the exp chain and the first matmuls start
                # after ~512KB instead of ~1MB
                for h in range(2):
                    nc.sync.dma_start(
                        a_t[:, h * 2 : (h + 1) * 2],
                        attsT[n][h * 256 : (h + 1) * 256].rearrange(
                            "(kd p) c -> p kd c", p=P
                        ),
                    )
                    nc.sync.dma_start(
                        x_t[:, h * 2 : (h + 1) * 2],
                        images[n][h * 256 : (h + 1) * 256].rearrange(
                            "(kd p) f -> p kd f", p=P
                        ),
                    )
            else:
                nc.sync.dma_start(
                    a_t[:], attsT[n].rearrange("(kd p) c -> p kd c", p=P)
                )
                nc.sync.dma_start(
                    x_t[:], images[n].rearrange("(kd p) f -> p kd f", p=P)
                )
            for kd in range(KD):
                nc.scalar.activation(
                    e_t[:, kd], a_t[:, kd], AF.Exp, bias=0.0, scale=1.0
                )
            return e_t, x_t

        def prep_T(n, e_t):
            """T[d_p, c] = sum_kd E[d_p, kd, c] (DVE): free-axis half of the
            softmax denominator; the partition half happens in tiny matmuls."""
            t2 = t_pool.tile([P, 2, C], F16, name=f"t2_{n}", tag="t2")
            nc.vector.scalar_tensor_tensor(
                t2[:, 0], e_t[:, 0], 1.0, e_t[:, 1], op0=OP.mult, op1=OP.add
            )
            nc.vector.scalar_tensor_tensor(
                t2[:, 1], e_t[:, 2], 1.0, e_t[:, 3], op0=OP.mult, op1=OP.add
            )
            tsum = t_pool.tile([P, C], F16, name=f"ts{n}", tag="ts")
            nc.vector.scalar_tensor_tensor(
                tsum[:], t2[:, 0], 1.0, t2[:, 1], op0=OP.mult, op1=OP.add
            )
            return tsum

        def denom(n, tsum):
            """s[c] = sum_p T[p, c] via tiny matmuls (lands the softmax
            denominator directly on the output-channel partitions); r = 1/s."""
            rp_ps = sm_psum.tile(
                [P, 2 * KC], F32, name=f"rp{n}", tag="rp", space="PSUM"
            )
            for j in range(KC):
                nc.tensor.matmul(
                    rp_ps[:, j * 2 : (j + 1) * 2],
                    lhsT=tsum[:, j * P : (j + 1) * P],
                    rhs=ones2[:],
                )
            s_col = r_pool.tile([P, KC], F32, name=f"scol{n}", tag="scol")
            nc.vector.tensor_copy(
                s_col[:],
                rp_ps[:].rearrange("p (kc j) -> p kc j", j=2)[:, :, 0],
            )
            r_sb = r_pool.tile([P, KC], F32, name=f"rsb{n}", tag="rsb")
            nc.vector.reciprocal(r_sb[:], s_col[:])
            return r_sb

        def mm_band(ps, e_t, x_t, kc, ht_list):
            for kd in range(KD):
                for ht in ht_list:
                    nc.tensor.matmul(
                        ps[:, ht * NT : (ht + 1) * NT],
                        lhsT=e_t[:, kd, kc * P : (kc + 1) * P],
                        rhs=x_t[:, kd, ht * NT : (ht + 1) * NT],
                        start=(kd == 0),
                        stop=(kd == KD - 1),
                    )

        def compute(n, e_t, x_t, tsum, next_io, next_T):
            """next_io/next_T: callbacks emitting the next sample's prep at
            queue positions that keep evictions ahead of dependent exps."""
            last = n == NPC - 1
            r_sb = None if n == 0 else denom(n, tsum)
            for kc in range(KC):
                ps = mm_psum.tile(
                    [P, HW], F32, name=f"ps{n}_{kc}", tag="ps", space="PSUM"
                )
                mm_band(ps, e_t, x_t, kc, range(NHT))
                if kc == 0 and n == 0:
                    r_sb = denom(n, tsum)
                o_t = o_pool.tile([P, HW], F16, name=f"o{n}_{kc}", tag=f"o{kc}")
                r_ap = r_sb[:, kc : kc + 1]
                if last and kc == KC - 1:
                    # tail: evict the final band in quarters — three on the
                    # fast-waking ACT, the last on DVE — with stores fanned
                    # across the gpsimd and (idle) sync queues, so the
                    # post-matmul critical path is one quarter-evict plus
                    # one 64KB store
                    NQ = HW // 4
                    for q in range(4):
                        sl = slice(q * NQ, (q + 1) * NQ)
                        if q < 3:
                            nc.scalar.mul(o_t[:, sl], ps[:, sl], r_ap)
                        else:
                            nc.vector.tensor_scalar_mul(
                                o_t[:, sl], ps[:, sl], r_ap
                            )
                        eng = nc.gpsimd if q % 2 == 0 else nc.sync
                        eng.dma_start(
                            out[n][kc * P : (kc + 1) * P, sl], o_t[:, sl]
                        )
                    continue
                # eviction engines: ACT on even kc, DVE on odd. For the last
                # sample, kc0 goes to the otherwise-idle DVE so the psum
                # slot the final band reuses is freed early, and the DVE is
                # kept clear of kc2 so the final split eviction isn't queued
                # behind a 1.3us whole-band eviction.
                act_evict = (kc % 2 == 0) if not last else (kc in (1, 2))
                if act_evict:
                    nc.scalar.mul(o_t[:], ps[:], r_ap)
                    nc.gpsimd.dma_start(out[n][kc * P : (kc + 1) * P], o_t[:])
                else:
                    nc.vector.tensor_scalar_mul(o_t[:], ps[:], r_ap)
                    nc.scalar.dma_start(out[n][kc * P : (kc + 1) * P], o_t[:])
                if kc == 0 and next_io is not None:
                    next_io()
                if kc == 1 and next_T is not None:
                    next_T()

        # software pipeline: sample n+1's loads/exps are emitted inside
        # compute(n) right after evict(kc0) so ACT never holds a PSUM bank
        # hostage behind a DMA-gated exp
        state = {}
        state[0] = prep_io(0)
        t0 = prep_T(0, state[0][0])

        def mk_io(m):
            def f():
                state[m] = prep_io(m)
            return f

        def mk_T(m):
            def f():
                state[m] = (*state[m], prep_T(m, state[m][0]))
            return f

        cur_T = t0
        for n in range(NPC):
            e_t, x_t = state[n][0], state[n][1]
            nio = mk_io(n + 1) if n + 1 < NPC else None
            nT = mk_T(n + 1) if n + 1 < NPC else None
            compute(n, e_t, x_t, cur_T, nio, nT)
            if n + 1 < NPC:
                cur_T = state[n + 1][2]

    nc.compile()
    return nc


_NC_CACHE = None


def _get_nc():
    global _NC_CACHE
    if _NC_CACHE is None:
        _NC_CACHE = build_nc()
    return _NC_CACHE


def run(in_maps, **kwargs):
    """Run the SPMD kernel on cores 0..7. in_maps: one dict per core."""
    nc = _get_nc()
    return run_bass_kernel_spmd(nc, in_maps, core_ids=list(range(NCORES)), **kwargs)


def make_in_maps(images: np.ndarray, atts: np.ndarray):
    images = np.asarray(images, dtype=np.float32).astype(np.float16)
    atts = np.asarray(atts, dtype=np.float32)
    assert images.shape == (N, C, H, W), images.shape
    assert atts.shape == (N, C, C), atts.shape
    img_s = images.reshape(NCORES, NPC, C, HW)
    # per-sample transpose: attsT[n] = atts[n].T  (layout [d, c])
    attsT = np.ascontiguousarray(atts.transpose(0, 2, 1)).astype(np.float16)
    attsT = attsT.reshape(NCORES, NPC, C, C)
    return [
        {"images": np.ascontiguousarray(img_s[i]), "attsT": attsT[i]}
        for i in range(NCORES)
    ]


def kernel(images: np.ndarray, atts: np.ndarray) -> np.ndarray:
    in_maps = make_in_maps(images, atts)
    res = run(in_maps)
    outs = [res.results[i]["out"] for i in range(NCORES)]
    full = np.concatenate(outs, axis=0).reshape(N, C, H, W)
    return full.astype(np.float32)


# revision 24
# speedup vs baseline: 1.0151x; 1.0080x over previous
"""Trainium2 Bass kernel for AttentionalPlanarRemapping.

out[n,c,h,w] = sum_d softmax(atts[n,c,:])[d] * images[n,d,h,w]

Per-sample: W = softmax(atts[n]) [C,C]; out[n] = W @ images[n].reshape(C, H*W).

Sharding: data-parallel over N across 8 cores (4 samples per core).

Host preprocessing inside kernel(): atts is passed TRANSPOSED per sample
(attsT[n] = atts[n].T, layout [d, c]) and converted to fp16, so attsT loads
with the contraction dim d on partitions (the matmul lhsT layout) at half
the DMA cost. images are uploaded fp16 and the output stored fp16: the
rel-err budget (2e-2) dwarfs fp16 rounding, and HBM bandwidth is the
co-bottleneck with the PE (10MB DMA ~28us vs 27.5us of fp16 matmul).

Per-core structure (pipelined one sample ahead):
  prep_io(n): one whole-sample DMA per tensor (the sync sequencer's
    DIRECT2D issue cost ~0.6us is per-dma_start, so chunked loads
    serialize on issue), then E = exp(attsT) per kd-block on ACT.
  prep_T(n):  T = sum_kd E (3 DVE adds) -- the free-axis half of the
    softmax denominator.
  compute(n): 4 tiny matmuls T_blk.T @ ones[128,2] finish the denominator
    (partition-sum) directly in per-partition layout; r = 1/s (DVE); then
    per kc: 8 accumulating matmuls into psum, evict psum -> O fp16 scaled
    by r[kc] (alternating ACT/DVE), store per kc band (alternating
    SWDGE/ACT-HWDGE queues so stores never block the sync load queue).
    The very last band's eviction is split ACT/DVE in parallel with its
    two half-stores on the gpsimd/sync queues (shortest post-matmul tail;
    PSUM dependencies are tile-granular, so an eviction can never overlap
    later matmuls into the same psum tile).

Emission interleaves prep(n+1) into compute(n) so the ACT queue runs
[evict(n,kc0), exps(n+1), evict(n,kc2)]: an eviction is never queued
behind exps that wait on a DMA, which would hold PSUM banks hostage and
stall the PE (engines execute their instruction streams in FIFO order).

No PE warmup stream: the HAM clock gate needs ~3.4us of CONTINUOUS PE
activity to lift, which cannot fit before the first loads land (~10us),
and a busy PE delays DMA completion semaphores ~1:1 (SBUF port
contention), starving the exp chain. The first ~10 matmuls run at
1.2GHz; that is unavoidable.
"""

import numpy as np
from contextlib import ExitStack

import concourse.bass as bass
import concourse.mybir as mybir
import concourse.tile as tile
from concourse import bacc
from concourse.bass_utils import run_bass_kernel_spmd

N, C, H, W = 32, 512, 32, 32
HW = H * W                      # 1024
NCORES = 8
NPC = N // NCORES               # 4 samples per core
P = 128
KC = C // P                     # 4 chunks over output channel c
KD = C // P                     # 4 chunks over contraction d
NT = 512                        # matmul moving free dim (one PSUM bank of f32)
NHT = HW // NT                  # 2

F32 = mybir.dt.float32
F16 = mybir.dt.float16
AF = mybir.ActivationFunctionType
OP = mybir.AluOpType


def build_nc():
    nc = bacc.Bacc("TRN2", target_bir_lowering=False, debug=False)

    images = nc.dram_tensor("images", [NPC, C, HW], F16, kind="ExternalInput").ap()
    attsT = nc.dram_tensor("attsT", [NPC, C, C], F16, kind="ExternalInput").ap()
    out = nc.dram_tensor("out", [NPC, C, HW], F16, kind="ExternalOutput").ap()

    with ExitStack() as ctx:
        tc = ctx.enter_context(tile.TileContext(nc))

        const_pool = ctx.enter_context(tc.tile_pool(name="const", bufs=1))
        ones2_f32 = const_pool.tile([P, 2], F32)
        ones2 = const_pool.tile([P, 2], F16)

        a_pool = ctx.enter_context(tc.tile_pool(name="a", bufs=2))
        e_pool = ctx.enter_context(tc.tile_pool(name="e", bufs=2))
        t_pool = ctx.enter_context(tc.tile_pool(name="t", bufs=2))
        x_pool = ctx.enter_context(tc.tile_pool(name="x", bufs=3))
        o_pool = ctx.enter_context(tc.tile_pool(name="o", bufs=2))
        r_pool = ctx.enter_context(tc.tile_pool(name="r", bufs=2))
        sm_psum = ctx.enter_context(tc.tile_pool(name="smp", bufs=1, space="PSUM"))
        mm_psum = ctx.enter_context(tc.tile_pool(name="mmp", bufs=3, space="PSUM"))

        nc.vector.memset(ones2_f32[:], 1.0)
        nc.vector.tensor_copy(ones2[:], ones2_f32[:])

        def prep_io(n):
            """Whole-sample input DMAs + exp for sample n."""
            a_t = a_pool.tile([P, KD, C], F16, name=f"a{n}", tag="a")
            x_t = x_pool.tile([P, KD, HW], F16, name=f"x{n}", tag="x")
            e_t = e_pool.tile([P, KD, C], F16, name=f"e{n}", tag="e")
            if n == 0:
                # entry is load-bandwidth-bound: interleave kd-halves of the
                # two tensors so the exp chain and the first matmuls start
                # after ~512KB instead of ~1MB
                for h in range(2):
                    nc.sync.dma_start(
                        a_t[:, h * 2 : (h + 1) * 2],
                        attsT[n][h * 256 : (h + 1) * 256].rearrange(
                            "(kd p) c -> p kd c", p=P
                        ),
                    )
                    nc.sync.dma_start(
                        x_t[:, h * 2 : (h + 1) * 2],
                        images[n][h * 256 : (h + 1) * 256].rearrange(
                            "(kd p) f -> p kd f", p=P
                        ),
                    )
            else:
                nc.sync.dma_start(
                    a_t[:], attsT[n].rearrange("(kd p) c -> p kd c", p=P)
                )
                nc.sync.dma_start(
                    x_t[:], images[n].rearrange("(kd p) f -> p kd f", p=P)
                )
            for kd in range(KD):
                nc.scalar.activation(
                    e_t[:, kd], a_t[:, kd], AF.Exp, bias=0.0, scale=1.0
                )
            return e_t, x_t

        def prep_T(n, e_t):
            """T[d_p, c] = sum_kd E[d_p, kd, c] (DVE): free-axis half of the
            softmax denominator; the partition half happens in tiny matmuls."""
            t2 = t_pool.tile([P, 2, C], F16, name=f"t2_{n}", tag="t2")
            nc.vector.scalar_tensor_tensor(
                t2[:, 0], e_t[:, 0], 1.0, e_t[:, 1], op0=OP.mult, op1=OP.add
            )
            nc.vector.scalar_tensor_tensor(
                t2[:, 1], e_t[:, 2], 1.0, e_t[:, 3], op0=OP.mult, op1=OP.add
            )
            tsum = t_pool.tile([P, C], F16, name=f"ts{n}", tag="ts")
            nc.vector.scalar_tensor_tensor(
                tsum[:], t2[:, 0], 1.0, t2[:, 1], op0=OP.mult, op1=OP.add
            )
            return tsum

        def denom(n, tsum):
            """s[c] = sum_p T[p, c] via tiny matmuls (lands the softmax
            denominator directly on the output-channel partitions); r = 1/s."""
            rp_ps = sm_psum.tile(
                [P, 2 * KC], F32, name=f"rp{n}", tag="rp", space="PSUM"
            )
            for j in range(KC):
                nc.tensor.matmul(
                    rp_ps[:, j * 2 : (j + 1) * 2],
                    lhsT=tsum[:, j * P : (j + 1) * P],
                    rhs=ones2[:],
                )
            s_col = r_pool.tile([P, KC], F32, name=f"scol{n}", tag="scol")
            nc.vector.tensor_copy(
                s_col[:],
                rp_ps[:].rearrange("p (kc j) -> p kc j", j=2)[:, :, 0],
            )
            r_sb = r_pool.tile([P, KC], F32, name=f"rsb{n}", tag="rsb")
            nc.vector.reciprocal(r_sb[:], s_col[:])
            return r_sb

        def mm_band(ps, e_t, x_t, kc, ht_list):
            for kd in range(KD):
                for ht in ht_list:
                    nc.tensor.matmul(
                        ps[:, ht * NT : (ht + 1) * NT],
                        lhsT=e_t[:, kd, kc * P : (kc + 1) * P],
                        rhs=x_t[:, kd, ht * NT : (ht + 1) * NT],
                        start=(kd == 0),
                        stop=(kd == KD - 1),
                    )

        def compute(n, e_t, x_t, tsum, next_io, next_T):
            """next_io/next_T: callbacks emitting the next sample's prep at
            queue positions that keep evictions ahead of dependent exps."""
            last = n == NPC - 1
            r_sb = None if n == 0 else denom(n, tsum)
            for kc in range(KC):
                ps = mm_psum.tile(
                    [P, HW], F32, name=f"ps{n}_{kc}", tag="ps", space="PSUM"
                )
                mm_band(ps, e_t, x_t, kc, range(NHT))
                if kc == 0 and n == 0:
                    r_sb = denom(n, tsum)
                o_t = o_pool.tile([P, HW], F16, name=f"o{n}_{kc}", tag=f"o{kc}")
                r_ap = r_sb[:, kc : kc + 1]
                if last and kc == KC - 1:
                    # tail: split the final eviction across ACT+DVE and put
                    # the final half-store on the idle sync queue; two
                    # 128KB stores beat four 64KB ones (per-DMA completion
                    # receipt dominates at that size)
                    nc.scalar.mul(o_t[:, :NT], ps[:, :NT], r_ap)
                    nc.vector.tensor_scalar_mul(o_t[:, NT:], ps[:, NT:], r_ap)
                    nc.gpsimd.dma_start(
                        out[n][kc * P : (kc + 1) * P, :NT], o_t[:, :NT]
                    )
                    nc.sync.dma_start(
                        out[n][kc * P : (kc + 1) * P, NT:], o_t[:, NT:]
                    )
                    continue
                # eviction engines: ACT on even kc, DVE on odd. For the last
                # sample, kc0 goes to the otherwise-idle DVE so the psum
                # slot the final band reuses is freed early, and the DVE is
                # kept clear of kc2 so the final split eviction isn't queued
                # behind a 1.3us whole-band eviction.
                act_evict = (kc % 2 == 0) if not last else (kc in (1, 2))
                if act_evict:
                    nc.scalar.mul(o_t[:], ps[:], r_ap)
                    nc.gpsimd.dma_start(out[n][kc * P : (kc + 1) * P], o_t[:])
                else:
                    nc.vector.tensor_scalar_mul(o_t[:], ps[:], r_ap)
                    nc.scalar.dma_start(out[n][kc * P : (kc + 1) * P], o_t[:])
                if kc == 0 and next_io is not None:
                    next_io()
                if kc == 1 and next_T is not None:
                    next_T()

        # software pipeline: sample n+1's loads/exps are emitted inside
        # compute(n) right after evict(kc0) so ACT never holds a PSUM bank
        # hostage behind a DMA-gated exp
        state = {}
        state[0] = prep_io(0)
        t0 = prep_T(0, state[0][0])

        def mk_io(m):
            def f():
                state[m] = prep_io(m)
            return f

        def mk_T(m):
            def f():
                state[m] = (*state[m], prep_T(m, state[m][0]))
            return f

        cur_T = t0
        for n in range(NPC):
            e_t, x_t = state[n][0], state[n][1]
            nio = mk_io(n + 1) if n + 1 < NPC else None
            nT = mk_T(n + 1) if n + 1 < NPC else None
            compute(n, e_t, x_t, cur_T, nio, nT)
            if n + 1 < NPC:
                cur_T = state[n + 1][2]

    nc.compile()
    return nc


_NC_CACHE = None


def _get_nc():
    global _NC_CACHE
    if _NC_CACHE is None:
        _NC_CACHE = build_nc()
    return _NC_CACHE


def run(in_maps, **kwargs):
    """Run the SPMD kernel on cores 0..7. in_maps: one dict per core."""
    nc = _get_nc()
    return run_bass_kernel_spmd(nc, in_maps, core_ids=list(range(NCORES)), **kwargs)


def make_in_maps(images: np.ndarray, atts: np.ndarray):
    images = np.asarray(images, dtype=np.float32).astype(np.float16)
    atts = np.asarray(atts, dtype=np.float32)
    assert images.shape == (N, C, H, W), images.shape
    assert atts.shape == (N, C, C), atts.shape
    img_s = images.reshape(NCORES, NPC, C, HW)
    # per-sample transpose: attsT[n] = atts[n].T  (layout [d, c])
    attsT = np.ascontiguousarray(atts.transpose(0, 2, 1)).astype(np.float16)
    attsT = attsT.reshape(NCORES, NPC, C, C)
    return [
        {"images": np.ascontiguousarray(img_s[i]), "attsT": attsT[i]}
        for i in range(NCORES)
    ]


def kernel(images: np.ndarray, atts: np.ndarray) -> np.ndarray:
    in_maps = make_in_maps(images, atts)
    res = run(in_maps)
    outs = [res.results[i]["out"] for i in range(NCORES)]
    full = np.concatenate(outs, axis=0).reshape(N, C, H, W)
    return full.astype(np.float32)


# revision 25
# speedup vs baseline: 1.0409x; 1.0255x over previous
"""Trainium2 Bass kernel for AttentionalPlanarRemapping.

out[n,c,h,w] = sum_d softmax(atts[n,c,:])[d] * images[n,d,h,w]

Per-sample: W = softmax(atts[n]) [C,C]; out[n] = W @ images[n].reshape(C, H*W).

Sharding: data-parallel over N across 8 cores (4 samples per core).

Host preprocessing inside kernel(): atts is passed TRANSPOSED per sample
(attsT[n] = atts[n].T, layout [d, c]) and converted to fp16, so attsT loads
with the contraction dim d on partitions (the matmul lhsT layout) at half
the DMA cost. images are uploaded fp16 and the output stored fp16: the
rel-err budget (2e-2) dwarfs fp16 rounding, and HBM bandwidth is the
co-bottleneck with the PE (10MB DMA ~28us vs 27.5us of fp16 matmul).

Per-core structure (pipelined one sample ahead):
  prep_io(n): one whole-sample DMA per tensor (the sync sequencer's
    DIRECT2D issue cost ~0.6us is per-dma_start, so chunked loads
    serialize on issue), then E = exp(attsT) per kd-block on ACT.
  prep_T(n):  T = sum_kd E (3 DVE adds) -- the free-axis half of the
    softmax denominator.
  compute(n): 4 tiny matmuls T_blk.T @ ones[128,2] finish the denominator
    (partition-sum) directly in per-partition layout; r = 1/s (DVE); then
    per kc: 8 accumulating matmuls into psum, evict psum -> O fp16 scaled
    by r[kc] (alternating ACT/DVE), store per kc band (alternating
    SWDGE/ACT-HWDGE queues so stores never block the sync load queue).
    The very last band's eviction is split ACT/DVE in parallel with its
    two half-stores on the gpsimd/sync queues (shortest post-matmul tail;
    PSUM dependencies are tile-granular, so an eviction can never overlap
    later matmuls into the same psum tile).

Emission interleaves prep(n+1) into compute(n) so the ACT queue runs
[evict(n,kc0), exps(n+1), evict(n,kc2)]: an eviction is never queued
behind exps that wait on a DMA, which would hold PSUM banks hostage and
stall the PE (engines execute their instruction streams in FIFO order).

No PE warmup stream: the HAM clock gate needs ~3.4us of CONTINUOUS PE
activity to lift, which cannot fit before the first loads land (~10us),
and a busy PE delays DMA completion semaphores ~1:1 (SBUF port
contention), starving the exp chain. The first ~10 matmuls run at
1.2GHz; that is unavoidable.
"""

import numpy as np
from contextlib import ExitStack

import concourse.bass as bass
import concourse.mybir as mybir
import concourse.tile as tile
from concourse import bacc
from concourse.bass_utils import run_bass_kernel_spmd

N, C, H, W = 32, 512, 32, 32
HW = H * W                      # 1024
NCORES = 8
NPC = N // NCORES               # 4 samples per core
P = 128
KC = C // P                     # 4 chunks over output channel c
KD = C // P                     # 4 chunks over contraction d
NT = 512                        # matmul moving free dim (one PSUM bank of f32)
NHT = HW // NT                  # 2

F32 = mybir.dt.float32
F16 = mybir.dt.float16
AF = mybir.ActivationFunctionType
OP = mybir.AluOpType


def build_nc():
    nc = bacc.Bacc("TRN2", target_bir_lowering=False, debug=False)

    images = nc.dram_tensor("images", [NPC, C, HW], F16, kind="ExternalInput").ap()
    attsT = nc.dram_tensor("attsT", [NPC, C, C], F16, kind="ExternalInput").ap()
    out = nc.dram_tensor("out", [NPC, C, HW], F16, kind="ExternalOutput").ap()

    with ExitStack() as ctx:
        tc = ctx.enter_context(tile.TileContext(nc))

        const_pool = ctx.enter_context(tc.tile_pool(name="const", bufs=1))
        ones2_f32 = const_pool.tile([P, 2], F32)
        ones2 = const_pool.tile([P, 2], F16)

        a_pool = ctx.enter_context(tc.tile_pool(name="a", bufs=2))
        e_pool = ctx.enter_context(tc.tile_pool(name="e", bufs=2))
        t_pool = ctx.enter_context(tc.tile_pool(name="t", bufs=2))
        x_pool = ctx.enter_context(tc.tile_pool(name="x", bufs=3))
        o_pool = ctx.enter_context(tc.tile_pool(name="o", bufs=2))
        r_pool = ctx.enter_context(tc.tile_pool(name="r", bufs=2))
        sm_psum = ctx.enter_context(tc.tile_pool(name="smp", bufs=1, space="PSUM"))
        mm_psum = ctx.enter_context(tc.tile_pool(name="mmp", bufs=3, space="PSUM"))

        nc.vector.memset(ones2_f32[:], 1.0)
        nc.vector.tensor_copy(ones2[:], ones2_f32[:])

        def prep_io(n):
            """Whole-sample input DMAs + exp for sample n."""
            a_t = a_pool.tile([P, KD, C], F16, name=f"a{n}", tag="a")
            x_t = x_pool.tile([P, KD, HW], F16, name=f"x{n}", tag="x")
            e_t = e_pool.tile([P, KD, C], F16, name=f"e{n}", tag="e")
            if n == 0:
                # entry is load-bandwidth-bound: interleave kd-halves of the
                # two tensors so the exp chain and the first matmuls start
                # after ~512KB instead of ~1MB
                for h in range(2):
                    nc.sync.dma_start(
                        a_t[:, h * 2 : (h + 1) * 2],
                        attsT[n][h * 256 : (h + 1) * 256].rearrange(
                            "(kd p) c -> p kd c", p=P
                        ),
                    )
                    nc.sync.dma_start(
                        x_t[:, h * 2 : (h + 1) * 2],
                        images[n][h * 256 : (h + 1) * 256].rearrange(
                            "(kd p) f -> p kd f", p=P
                        ),
                    )
            else:
                nc.sync.dma_start(
                    a_t[:], attsT[n].rearrange("(kd p) c -> p kd c", p=P)
                )
                nc.sync.dma_start(
                    x_t[:], images[n].rearrange("(kd p) f -> p kd f", p=P)
                )
            for kd in range(KD):
                nc.scalar.activation(
                    e_t[:, kd], a_t[:, kd], AF.Exp, bias=0.0, scale=1.0
                )
            return e_t, x_t

        def prep_T(n, e_t):
            """T[d_p, c] = sum_kd E[d_p, kd, c] (DVE): free-axis half of the
            softmax denominator; the partition half happens in tiny matmuls."""
            t2 = t_pool.tile([P, 2, C], F16, name=f"t2_{n}", tag="t2")
            nc.vector.scalar_tensor_tensor(
                t2[:, 0], e_t[:, 0], 1.0, e_t[:, 1], op0=OP.mult, op1=OP.add
            )
            nc.vector.scalar_tensor_tensor(
                t2[:, 1], e_t[:, 2], 1.0, e_t[:, 3], op0=OP.mult, op1=OP.add
            )
            tsum = t_pool.tile([P, C], F16, name=f"ts{n}", tag="ts")
            nc.vector.scalar_tensor_tensor(
                tsum[:], t2[:, 0], 1.0, t2[:, 1], op0=OP.mult, op1=OP.add
            )
            return tsum

        def denom(n, tsum):
            """s[c] = sum_p T[p, c] via tiny matmuls (lands the softmax
            denominator directly on the output-channel partitions); r = 1/s."""
            rp_ps = sm_psum.tile(
                [P, 2 * KC], F32, name=f"rp{n}", tag="rp", space="PSUM"
            )
            for j in range(KC):
                nc.tensor.matmul(
                    rp_ps[:, j * 2 : (j + 1) * 2],
                    lhsT=tsum[:, j * P : (j + 1) * P],
                    rhs=ones2[:],
                )
            s_col = r_pool.tile([P, KC], F32, name=f"scol{n}", tag="scol")
            nc.vector.tensor_copy(
                s_col[:],
                rp_ps[:].rearrange("p (kc j) -> p kc j", j=2)[:, :, 0],
            )
            r_sb = r_pool.tile([P, KC], F32, name=f"rsb{n}", tag="rsb")
            nc.vector.reciprocal(r_sb[:], s_col[:])
            return r_sb

        def mm_band(ps, e_t, x_t, kc, ht_list):
            for kd in range(KD):
                for ht in ht_list:
                    nc.tensor.matmul(
                        ps[:, ht * NT : (ht + 1) * NT],
                        lhsT=e_t[:, kd, kc * P : (kc + 1) * P],
                        rhs=x_t[:, kd, ht * NT : (ht + 1) * NT],
                        start=(kd == 0),
                        stop=(kd == KD - 1),
                    )

        def compute(n, e_t, x_t, tsum, next_io, next_T):
            """next_io/next_T: callbacks emitting the next sample's prep at
            queue positions that keep evictions ahead of dependent exps."""
            last = n == NPC - 1
            r_sb = None if n == 0 else denom(n, tsum)
            for kc in range(KC):
                if last and kc == KC - 1:
                    # tail: the final band's ht0 gets its own 1-bank psum
                    # tile, so (PSUM deps being tile-granular) its eviction
                    # and store overlap ht1's matmuls; only one half-band
                    # eviction + one 128KB store remain after the very last
                    # matmul. Both evictions on ACT (wakes in ~50ns; DVE
                    # showed ~0.85us wakeup lag), stores on gpsimd + idle
                    # sync queues.
                    r_ap = r_sb[:, kc : kc + 1]
                    o_t = o_pool.tile(
                        [P, HW], F16, name=f"o{n}_{kc}", tag=f"o{kc}"
                    )
                    ps0 = sm_psum.tile(
                        [P, NT], F32, name="ps_bh0", tag="bh", space="PSUM"
                    )
                    for kd in range(KD):
                        nc.tensor.matmul(
                            ps0[:],
                            lhsT=e_t[:, kd, kc * P : (kc + 1) * P],
                            rhs=x_t[:, kd, :NT],
                            start=(kd == 0),
                            stop=(kd == KD - 1),
                        )
                    nc.scalar.mul(o_t[:, :NT], ps0[:], r_ap)
                    nc.gpsimd.dma_start(
                        out[n][kc * P : (kc + 1) * P, :NT], o_t[:, :NT]
                    )
                    ps = mm_psum.tile(
                        [P, HW], F32, name=f"ps{n}_{kc}", tag="ps", space="PSUM"
                    )
                    for kd in range(KD):
                        nc.tensor.matmul(
                            ps[:, :NT],
                            lhsT=e_t[:, kd, kc * P : (kc + 1) * P],
                            rhs=x_t[:, kd, NT:],
                            start=(kd == 0),
                            stop=(kd == KD - 1),
                        )
                    nc.scalar.mul(o_t[:, NT:], ps[:, :NT], r_ap)
                    nc.sync.dma_start(
                        out[n][kc * P : (kc + 1) * P, NT:], o_t[:, NT:]
                    )
                    continue
                ps = mm_psum.tile(
                    [P, HW], F32, name=f"ps{n}_{kc}", tag="ps", space="PSUM"
                )
                mm_band(ps, e_t, x_t, kc, range(NHT))
                if kc == 0 and n == 0:
                    r_sb = denom(n, tsum)
                o_t = o_pool.tile([P, HW], F16, name=f"o{n}_{kc}", tag=f"o{kc}")
                r_ap = r_sb[:, kc : kc + 1]
                # eviction engines: ACT on even kc, DVE on odd. For the last
                # sample, kc0 goes to the otherwise-idle DVE so the psum
                # slot the final band reuses is freed early, and the DVE is
                # kept clear of kc2 so the final split eviction isn't queued
                # behind a 1.3us whole-band eviction.
                act_evict = (kc % 2 == 0) if not last else (kc in (1, 2))
                if act_evict:
                    nc.scalar.mul(o_t[:], ps[:], r_ap)
                    nc.gpsimd.dma_start(out[n][kc * P : (kc + 1) * P], o_t[:])
                else:
                    nc.vector.tensor_scalar_mul(o_t[:], ps[:], r_ap)
                    nc.scalar.dma_start(out[n][kc * P : (kc + 1) * P], o_t[:])
                if kc == 0 and next_io is not None:
                    next_io()
                if kc == 1 and next_T is not None:
                    next_T()

        # software pipeline: sample n+1's loads/exps are emitted inside
        # compute(n) right after evict(kc0) so ACT never holds a PSUM bank
        # hostage behind a DMA-gated exp
        state = {}
        state[0] = prep_io(0)
        t0 = prep_T(0, state[0][0])

        def mk_io(m):
            def f():
                state[m] = prep_io(m)
            return f

        def mk_T(m):
            def f():
                state[m] = (*state[m], prep_T(m, state[m][0]))
            return f

        cur_T = t0
        for n in range(NPC):
            e_t, x_t = state[n][0], state[n][1]
            nio = mk_io(n + 1) if n + 1 < NPC else None
            nT = mk_T(n + 1) if n + 1 < NPC else None
            compute(n, e_t, x_t, cur_T, nio, nT)
            if n + 1 < NPC:
                cur_T = state[n + 1][2]

    nc.compile()
    return nc


_NC_CACHE = None


def _get_nc():
    global _NC_CACHE
    if _NC_CACHE is None:
        _NC_CACHE = build_nc()
    return _NC_CACHE


def run(in_maps, **kwargs):
    """Run the SPMD kernel on cores 0..7. in_maps: one dict per core."""
    nc = _get_nc()
    return run_bass_kernel_spmd(nc, in_maps, core_ids=list(range(NCORES)), **kwargs)


def make_in_maps(images: np.ndarray, atts: np.ndarray):
    images = np.asarray(images, dtype=np.float32).astype(np.float16)
    atts = np.asarray(atts, dtype=np.float32)
    assert images.shape == (N, C, H, W), images.shape
    assert atts.shape == (N, C, C), atts.shape
    img_s = images.reshape(NCORES, NPC, C, HW)
    # per-sample transpose: attsT[n] = atts[n].T  (layout [d, c])
    attsT = np.ascontiguousarray(atts.transpose(0, 2, 1)).astype(np.float16)
    attsT = attsT.reshape(NCORES, NPC, C, C)
    return [
        {"images": np.ascontiguousarray(img_s[i]), "attsT": attsT[i]}
        for i in range(NCORES)
    ]


def kernel(images: np.ndarray, atts: np.ndarray) -> np.ndarray:
    in_maps = make_in_maps(images, atts)
    res = run(in_maps)
    outs = [res.results[i]["out"] for i in range(NCORES)]
    full = np.concatenate(outs, axis=0).reshape(N, C, H, W)
    return full.astype(np.float32)
